# revision 6
# baseline (speedup 1.0000x reference)
"""Bahdanau attention Trainium2 kernel (8-core SPMD, data-parallel over batch).

Full-input contract: kernel(**inputs) takes the unsharded tensors
(query [64,512], key [64,2048,512], value [64,2048,512], weights) and
returns (context [64,512], attn [64,2048]) matching the reference.

Per-core work (B_LOC=8 batches):
  phase A: stream key, transpose k-chunks on the PE, k_proj = key@Wk (fp32r
           matmuls), tanh(q_proj + k_proj + bq + bk) on ScalarE, score via a
           matmul against Wo broadcast over 8 columns so batch b's score row
           lands on PSUM partition b.
  softmax: one batched [8, 2048] pass (max/exp+accum/reciprocal/scale).
           bo is dropped: softmax is invariant to a constant shift.
  phase B: transpose attn to [t, 8] chunks, context = attnT.T @ value with
           accumulating matmuls, row b of each PSUM tile is batch b's context.
"""

import numpy as np

import concourse.bass as bass  # noqa: F401  (engine namespaces live on the Bacc object)
import concourse.mybir as mybir
import concourse.tile as tile
from concourse import bacc
from concourse.bass_utils import run_bass_kernel_spmd
from concourse.masks import make_identity

N_CORES = 8
B, T, KD, VD, AD = 64, 2048, 512, 512, 256
BL = B // N_CORES  # 8 batches per core

F32 = mybir.dt.float32
F32R = mybir.dt.float32r

P = 128
KC = KD // P  # 4 contraction chunks
AH = AD // P  # 2 attention-dim halves
TCH = 512  # t-chunk processed per inner iteration
NT = T // TCH  # 4 t-chunks per batch
NSUB = TCH // P  # 4 128-row blocks per t-chunk

ACT = mybir.ActivationFunctionType


def build():
    nc = bacc.Bacc(
        "TRN2", target_bir_lowering=False, debug=False, num_devices=N_CORES
    )
    query_d = nc.declare_dram_parameter("query", [BL, KD], F32, isOutput=False)
    key_d = nc.declare_dram_parameter("key", [BL, T, KD], F32, isOutput=False)
    value_d = nc.declare_dram_parameter("value", [BL, T, VD], F32, isOutput=False)
    wq_d = nc.declare_dram_parameter("Wq", [KD, AD], F32, isOutput=False)
    bq_d = nc.declare_dram_parameter("bq", [AD], F32, isOutput=False)
    wk_d = nc.declare_dram_parameter("Wk", [KD, AD], F32, isOutput=False)
    bk_d = nc.declare_dram_parameter("bk", [AD], F32, isOutput=False)
    wo_d = nc.declare_dram_parameter("Wo", [AD, 1], F32, isOutput=False)
    ctx_d = nc.declare_dram_parameter("context", [BL, VD], F32, isOutput=True)
    attn_d = nc.declare_dram_parameter("attn", [BL, T], F32, isOutput=True)

    with tile.TileContext(nc) as tc:
        with (
            tc.tile_pool(name="consts", bufs=1) as consts,
            tc.tile_pool(name="stream", bufs=3) as stream,
            tc.tile_pool(name="work", bufs=2) as work,
            tc.tile_pool(name="psA", bufs=1, space="PSUM") as psA,
            tc.tile_pool(name="psB", bufs=1, space="PSUM") as psB,
            tc.tile_pool(name="psC", bufs=2, space="PSUM") as psC,
        ):
            ident = consts.tile([P, P], F32, tag="ident")
            make_identity(nc, ident[:])

            wk_sb = consts.tile([P, KC, AD], F32R, tag="wk")
            nc.sync.dma_start(wk_sb[:], wk_d.rearrange("(c p) a -> p c a", p=P).bitcast(F32R))
            wq_sb = consts.tile([P, KC, AD], F32R, tag="wq")
            nc.sync.dma_start(wq_sb[:], wq_d.rearrange("(c p) a -> p c a", p=P).bitcast(F32R))

            bq_sb = consts.tile([P, AH], F32, tag="bq")
            nc.sync.dma_start(bq_sb[:], bq_d.rearrange("(c p) -> p c", p=P))
            bk_sb = consts.tile([P, AH], F32, tag="bk")
            nc.sync.dma_start(bk_sb[:], bk_d.rearrange("(c p) -> p c", p=P))
            bqk_sb = consts.tile([P, AH], F32, tag="bqk")
            nc.vector.tensor_add(bqk_sb[:], bq_sb[:], bk_sb[:])

            # One-hot Wo: wo_oh[p, ah, b, m] = Wo[ah*P+p] iff m == b. Using it as
            # lhsT for batch b's score matmul lands the score on PSUM row b, so
            # all 8 batches accumulate into one [8, TCH] tile per t-chunk.
            wo_sb = consts.tile([P, AH], F32, tag="wo")
            nc.sync.dma_start(wo_sb[:], wo_d.rearrange("(c p) o -> p (c o)", p=P))
            wo_stage = consts.tile([P, AH, BL, BL], F32, tag="wo_stage")
            nc.any.memzero(wo_stage[:])
            for c in range(AH):
                for b in range(BL):
                    nc.vector.tensor_copy(
                        wo_stage[:, c, b, b : b + 1], wo_sb[:, c : c + 1]
                    )
            wo_oh = consts.tile([P, AH, BL, BL], F32R, tag="wo_oh")
            nc.vector.tensor_copy(wo_oh[:], wo_stage[:])

            # queryT [KD on partitions, BL] then q_projT [AD on partitions, BL];
            # bias_sb[a, b] = q_proj[b, a] + bq[a] + bk[a]
            q_sb = consts.tile([BL, KD], F32, tag="q")
            nc.sync.dma_start(q_sb[:], query_d[:])
            qT_sb = consts.tile([P, KC, BL], F32R, tag="qT")
            for c in range(KC):
                pt = psA.tile([P, TCH], F32, tag="keyT0")
                nc.tensor.transpose(
                    pt[:, :BL], q_sb[:, c * P : (c + 1) * P], ident[:BL, :BL]
                )
                nc.vector.tensor_copy(qT_sb[:, c, :], pt[:, :BL])
            bias_sb = consts.tile([P, AH, BL], F32, tag="bias")
            for ah in range(AH):
                pq = psB.tile([P, TCH], F32, tag="kp0")
                for c in range(KC):
                    nc.tensor.matmul(
                        pq[:, :BL],
                        wq_sb[:, c, ah * P : (ah + 1) * P],
                        qT_sb[:, c, :],
                        start=(c == 0),
                        stop=(c == KC - 1),
                    )
                nc.scalar.activation(
                    bias_sb[:, ah, :],
                    pq[:, :BL],
                    ACT.Identity,
                    bias=bqk_sb[:, ah : ah + 1],
                    scale=1.0,
                )

            score_sb = consts.tile([BL, T], F32, tag="score")

            # ---- phase A: key -> scores ----
            for t in range(NT):
                ps = psC.tile([BL, TCH], F32, tag="bs512")
                for b in range(BL):
                    key_tile = stream.tile([P, NSUB, KD], F32, tag="key")
                    nc.sync.dma_start(
                        key_tile[:],
                        key_d[b, t * TCH : (t + 1) * TCH, :].rearrange(
                            "(n p) k -> p n k", p=P
                        ),
                    )
                    keyT = []
                    for c in range(KC):
                        pt = psA.tile([P, TCH], F32, tag=f"keyT{c}")
                        for n in range(NSUB):
                            nc.tensor.transpose(
                                pt[:, n * P : (n + 1) * P],
                                key_tile[:, n, c * P : (c + 1) * P],
                                ident[:],
                            )
                        kt = work.tile([P, TCH], F32R, tag=f"keyT{c}")
                        if c < 2:
                            nc.vector.tensor_copy(kt[:], pt[:])
                        else:
                            nc.scalar.copy(kt[:], pt[:])
                        keyT.append(kt)
                    tanh_tiles = []
                    for ah in range(AH):
                        pk = psB.tile([P, TCH], F32, tag=f"kp{ah}")
                        for c in range(KC):
                            nc.tensor.matmul(
                                pk[:],
                                wk_sb[:, c, ah * P : (ah + 1) * P],
                                keyT[c][:],
                                start=(c == 0),
                                stop=(c == KC - 1),
                            )
                        tnh = work.tile([P, TCH], F32R, tag=f"tanh{ah}")
                        nc.scalar.activation(
                            tnh[:],
                            pk[:],
                            ACT.Tanh,
                            bias=bias_sb[:, ah, b : b + 1],
                            scale=1.0,
                        )
                        tanh_tiles.append(tnh)
                    for ah in range(AH):
                        nc.tensor.matmul(
                            ps[:],
                            wo_oh[:, ah, b, :],
                            tanh_tiles[ah][:],
                            start=(b == 0 and ah == 0),
                            stop=(b == BL - 1 and ah == AH - 1),
                        )
                nc.vector.tensor_copy(
                    score_sb[:, t * TCH : (t + 1) * TCH], ps[:]
                )

            # ---- softmax over t, all 8 batches at once ----
            neg_max = consts.tile([BL, 1], F32, tag="negmax")
            nc.vector.reduce_max(
                neg_max[:], score_sb[:], axis=mybir.AxisListType.X, negate=True
            )
            attn_sb = consts.tile([BL, T], F32, tag="attn")
            sumexp = consts.tile([BL, 1], F32, tag="sumexp")
            nc.scalar.activation(
                attn_sb[:],
                score_sb[:],
                ACT.Exp,
                bias=neg_max[:],
                scale=1.0,
                accum_out=sumexp[:],
            )
            rsum = consts.tile([BL, 1], F32, tag="rsum")
            nc.vector.reciprocal(rsum[:], sumexp[:])
            nc.vector.tensor_scalar_mul(attn_sb[:], attn_sb[:], rsum[:])
            nc.sync.dma_start(attn_d[:], attn_sb[:])

            # attnT[t % P, t // P, b] = attn[b, t]
            attnT = consts.tile([P, T // P, BL], F32R, tag="attnT")
            for n in range(T // P):
                pt = psA.tile([P, TCH], F32, tag="keyT0")
                nc.tensor.transpose(
                    pt[:, :BL], attn_sb[:, n * P : (n + 1) * P], ident[:BL, :BL]
                )
                nc.vector.tensor_copy(attnT[:, n, :], pt[:, :BL])

            # ---- phase B: value -> context ----
            for b in range(BL):
                pc = psC.tile([BL, VD], F32, tag="bs512")
                for t in range(NT):
                    val_tile = stream.tile([P, NSUB, VD], F32R, tag="val")
                    nc.sync.dma_start(
                        val_tile[:],
                        value_d[b, t * TCH : (t + 1) * TCH, :]
                        .rearrange("(n p) v -> p n v", p=P)
                        .bitcast(F32R),
                    )
                    for n in range(NSUB):
                        nc.tensor.matmul(
                            pc[:],
                            attnT[:, t * NSUB + n, :],
                            val_tile[:, n, :],
                            start=(t == 0 and n == 0),
                            stop=(t == NT - 1 and n == NSUB - 1),
                        )
                ctmp = work.tile([BL, VD], F32, tag="ctmp")
                nc.vector.tensor_copy(ctmp[:], pc[:])
                nc.sync.dma_start(ctx_d[b : b + 1, :], ctmp[b : b + 1, :])

    nc.compile()
    return nc


_NC_CACHE = []


def _get_nc():
    if not _NC_CACHE:
        _NC_CACHE.append(build())
    return _NC_CACHE[0]


def run(inputs, trace=False, **spmd_kwargs):
    nc = _get_nc()
    in_maps = []
    for c in range(N_CORES):
        s = slice(c * BL, (c + 1) * BL)
        in_maps.append(
            {
                "query": np.ascontiguousarray(inputs["query"][s]),
                "key": np.ascontiguousarray(inputs["key"][s]),
                "value": np.ascontiguousarray(inputs["value"][s]),
                "Wq": np.asarray(inputs["Wq"]),
                "bq": np.asarray(inputs["bq"]),
                "Wk": np.asarray(inputs["Wk"]),
                "bk": np.asarray(inputs["bk"]),
                "Wo": np.asarray(inputs["Wo"]),
            }
        )
    res = run_bass_kernel_spmd(
        nc, in_maps, list(range(N_CORES)), trace=trace, **spmd_kwargs
    )
    context = np.concatenate([res.results[c]["context"] for c in range(N_CORES)], 0)
    attn = np.concatenate([res.results[c]["attn"] for c in range(N_CORES)], 0)
    return (context.astype(np.float32), attn.astype(np.float32)), res


def kernel(**inputs):
    out, _ = run(inputs, trace=False)
    return out
